# revision 3
# baseline (speedup 1.0000x reference)
"""AgentNet GNN message-passing kernel for 8 Trainium2 NeuronCores.

Algebraic collapse (validated to 4e-7 rel err vs reference in fp32,
8.9e-5 with the bf16 edge path used here):
  hidden = relu(obs @ enc_w + enc_b)                      [N,128]
  q'     = hidden @ Wq2 + bq2      (q/k projections folded; key bias
                                    cancels in softmax)   [N,128]
  s[n,k] = q'[n] . hidden[idx[n,k]]
  attn   = softmax_k(s)            (scores are O(0.05) -> no max-subtract)
  hctx   = sum_k attn[n,k] * hidden[idx[n,k]]
  logits = hidden @ dec_w + hctx @ W2 + b2
  out    = softmax_a(logits)

Key transformation: the neighbor gather commutes with the row-wise encoder,
so hidden[idx[n,k]] = relu(obs[idx[n,k]] @ enc_w + enc_b). The host gathers
obs rows per edge while sharding (cheap numpy take) and the device runs a
dense per-edge encoder matmul — no data-dependent DMA on device at all.

Sharding: nodes split into 8 contiguous shards (6250/core, padded to 6656).
Weights replicated; no collectives.
"""
import numpy as np

N, K = 50000, 16
IN_DIM, H, C, A = 64, 128, 128, 32
NCORES = 8
SHARD = N // NCORES              # 6250
P = 128
NSUP = 52                        # supertiles of 128 nodes per core (6656)
NS = NSUP * P                    # 6656 padded shard nodes
CSUP = 2                         # supertiles per chunk
NCHUNK = NSUP // CSUP            # 26
EPC = CSUP * P * K               # 4096 edges per chunk
CPP = EPC // P                   # 32 slots per partition per chunk

_PROG_CACHE = {}


def _build_program():
    import concourse.bacc as bacc
    import concourse.mybir as mybir
    import concourse.tile as tile
    from concourse.masks import make_identity

    f32 = mybir.dt.float32
    bf16 = mybir.dt.bfloat16
    AX = mybir.AxisListType
    OP = mybir.AluOpType
    AF = mybir.ActivationFunctionType

    nc = bacc.Bacc("TRN2", target_bir_lowering=False, debug=False, num_devices=1)

    # per-edge pre-gathered obs, transposed, with a trailing ones-row for the
    # encoder bias fold; column order: chunk j | slot u = g*16+k | partition p
    obsgT = nc.dram_tensor("obsgT", [IN_DIM + 1, NS * K], bf16, kind="ExternalInput")
    obsT_shard = nc.dram_tensor("obsT_shard", [IN_DIM + 1, NS], f32, kind="ExternalInput")
    enc_wb = nc.dram_tensor("enc_wb", [IN_DIM + 1, H], bf16, kind="ExternalInput")
    enc_wf = nc.dram_tensor("enc_wf", [IN_DIM + 1, H], f32, kind="ExternalInput")
    wq2 = nc.dram_tensor("wq2", [H, C], f32, kind="ExternalInput")
    bq2row = nc.dram_tensor("bq2row", [1, C], f32, kind="ExternalInput")
    dec_w = nc.dram_tensor("dec_w", [H, A], f32, kind="ExternalInput")
    w2 = nc.dram_tensor("w2", [C, A], f32, kind="ExternalInput")
    b2row = nc.dram_tensor("b2row", [1, A], f32, kind="ExternalInput")
    outp = nc.dram_tensor("outp", [P, NSUP * A], f32, kind="ExternalOutput")

    with tile.TileContext(nc) as tc:
        with tc.tile_pool(name="cst", bufs=1) as cst, \
             tc.tile_pool(name="obs", bufs=3) as obsp, \
             tc.tile_pool(name="psA", bufs=3, space="PSUM") as psA, \
             tc.tile_pool(name="big", bufs=1) as big, \
             tc.tile_pool(name="psm", bufs=2, space="PSUM") as psm, \
             tc.tile_pool(name="psQ", bufs=1, space="PSUM") as psQ, \
             tc.tile_pool(name="gat", bufs=3) as gat, \
             tc.tile_pool(name="pw", bufs=4) as pwp, \
             tc.tile_pool(name="sml", bufs=3) as sml, \
             tc.tile_pool(name="psL", bufs=2, space="PSUM") as psL:

            # ---- constants ----
            encwb_sb = cst.tile([IN_DIM + 1, H], bf16)
            nc.sync.dma_start(out=encwb_sb[:], in_=enc_wb[:, :])
            encwf_sb = cst.tile([IN_DIM + 1, H], f32)
            nc.sync.dma_start(out=encwf_sb[:], in_=enc_wf[:, :])
            wq2_sb = cst.tile([H, C], f32)
            nc.sync.dma_start(out=wq2_sb[:], in_=wq2[:, :])
            bq2_sb = cst.tile([1, C], f32)
            nc.sync.dma_start(out=bq2_sb[:], in_=bq2row[:, :])
            decw_sb = cst.tile([H, A], f32)
            nc.sync.dma_start(out=decw_sb[:], in_=dec_w[:, :])
            w2_sb = cst.tile([C, A], f32)
            nc.sync.dma_start(out=w2_sb[:], in_=w2[:, :])
            b2_sb = cst.tile([1, A], f32)
            nc.sync.dma_start(out=b2_sb[:], in_=b2row[:, :])
            ones1 = cst.tile([1, P], f32)
            nc.vector.memset(ones1[:], 1.0)
            ident = cst.tile([P, P], f32)
            make_identity(nc, ident[:])
            obss_sb = cst.tile([IN_DIM + 1, NS], f32)
            nc.sync.dma_start(out=obss_sb[:], in_=obsT_shard[:, :])

            # ---- stage B: shard hiddenT (f32) and q' (bf16) ----
            hidT = big.tile([H, NS], f32)
            for cb in range(NS // 512):
                ph = psm.tile([H, 512], f32, tag="mix", space="PSUM")
                nc.tensor.matmul(
                    out=ph[:], lhsT=encwf_sb[:],
                    rhs=obss_sb[:, cb * 512:(cb + 1) * 512],
                    start=True, stop=True)
                nc.scalar.activation(
                    out=hidT[:, cb * 512:(cb + 1) * 512], in_=ph[:], func=AF.Relu)

            qp = big.tile([P, NSUP * C], bf16)
            for s in range(NSUP):
                pq = psQ.tile([P, C], f32, tag="pq", space="PSUM")
                nc.tensor.matmul(out=pq[:], lhsT=hidT[:, s * P:(s + 1) * P],
                                 rhs=wq2_sb[:], start=True, stop=False)
                nc.tensor.matmul(out=pq[:], lhsT=ones1[:], rhs=bq2_sb[:],
                                 start=False, stop=True)
                nc.scalar.copy(out=qp[:, s * C:(s + 1) * C], in_=pq[:])

            # ---- stage C: per-edge encoder + attention + decode ----
            qp4 = qp[:].rearrange("p (s c) -> p s c", c=C)
            for j in range(NCHUNK):
                og = obsp.tile([IN_DIM + 1, EPC], bf16, tag="og")
                nc.sync.dma_start(
                    out=og[:, :EPC // 2],
                    in_=obsgT[:, j * EPC:j * EPC + EPC // 2])
                nc.sync.dma_start(
                    out=og[:, EPC // 2:],
                    in_=obsgT[:, j * EPC + EPC // 2:(j + 1) * EPC])

                g = gat.tile([P, CPP, H], bf16, tag="g")
                for u4 in range(CPP // 4):
                    pe = psA.tile([P, 4 * H], f32, tag="pe", space="PSUM")
                    for t in range(4):
                        u = u4 * 4 + t
                        nc.tensor.matmul(
                            out=pe[:, t * H:(t + 1) * H],
                            lhsT=og[:, u * P:(u + 1) * P],
                            rhs=encwb_sb[:], start=True, stop=True)
                    nc.scalar.activation(
                        out=g[:, u4 * 4:(u4 + 1) * 4, :], in_=pe[:], func=AF.Relu)

                g4 = g[:].rearrange("p (s k) c -> p s k c", k=K)
                qb = qp4[:, CSUP * j:CSUP * (j + 1), None, :].to_broadcast(
                    [P, CSUP, K, C])

                prod = pwp.tile([P, EPC], bf16, tag="pw")
                # scores product split: GPSIMD takes 12 slots of supertile 0,
                # DVE the rest (jointly swept optimum with the wp split below)
                pr4d = prod[:].rearrange("p (s k c) -> p s k c", k=K, c=C)
                nc.gpsimd.tensor_tensor(
                    out=pr4d[:, 0:1, :12, :], in0=g4[:, 0:1, :12, :],
                    in1=qb[:, 0:1, :12, :], op=OP.mult)
                nc.vector.tensor_tensor(
                    out=pr4d[:, 0:1, 12:, :], in0=g4[:, 0:1, 12:, :],
                    in1=qb[:, 0:1, 12:, :], op=OP.mult)
                nc.vector.tensor_tensor(
                    out=pr4d[:, 1:2, :, :], in0=g4[:, 1:2, :, :],
                    in1=qb[:, 1:2, :, :], op=OP.mult)

                # scores: tree-halving TT adds over c (bf16 2x mode) instead of
                # a 1x tensor_reduce; final level emits f32
                p3 = prod[:].rearrange("p (u c) -> p u c", c=C)
                # scores tree L1 split: GPSIMD takes 8 of 32 slots (swept)
                nc.gpsimd.tensor_tensor(
                    out=p3[:, :8, :64], in0=p3[:, :8, :64],
                    in1=p3[:, :8, 64:128], op=OP.add)
                nc.vector.tensor_tensor(
                    out=p3[:, 8:, :64], in0=p3[:, 8:, :64],
                    in1=p3[:, 8:, 64:128], op=OP.add)
                nc.gpsimd.tensor_tensor(
                    out=p3[:, :6, :32], in0=p3[:, :6, :32],
                    in1=p3[:, :6, 32:64], op=OP.add)
                nc.vector.tensor_tensor(
                    out=p3[:, 6:, :32], in0=p3[:, 6:, :32],
                    in1=p3[:, 6:, 32:64], op=OP.add)
                w = C // 8
                while w > 1:
                    nc.vector.tensor_tensor(
                        out=p3[:, :, :w], in0=p3[:, :, :w],
                        in1=p3[:, :, w:2 * w], op=OP.add)
                    w //= 2
                sc = sml.tile([P, CSUP * K], f32, tag="sc")
                nc.vector.tensor_tensor(
                    out=sc[:, :, None], in0=p3[:, :, 0:1], in1=p3[:, :, 1:2],
                    op=OP.add)

                esc = sml.tile([P, CSUP * K], f32, tag="esc")
                nc.scalar.activation(out=esc[:], in_=sc[:], func=AF.Exp)

                z = sml.tile([P, CSUP], f32, tag="z")
                nc.vector.reduce_sum(
                    out=z[:], in_=esc[:].rearrange("p (s k) -> p s k", k=K),
                    axis=AX.X)
                rz = sml.tile([P, CSUP], f32, tag="rz")
                nc.vector.reciprocal(out=rz[:], in_=z[:])

                attn = sml.tile([P, CSUP * K], bf16, tag="attn")
                nc.vector.tensor_tensor(
                    out=attn[:].rearrange("p (s k) -> p s k", k=K),
                    in0=esc[:].rearrange("p (s k) -> p s k", k=K),
                    in1=rz[:, :, None].to_broadcast([P, CSUP, K]),
                    op=OP.mult)

                wp = pwp.tile([P, EPC], bf16, tag="pw")
                # attn-weighting split across GPSIMD (idle) and DVE
                wp3 = wp[:].rearrange("p (u c) -> p u c", c=C)
                US = 11
                nc.gpsimd.tensor_tensor(
                    out=wp3[:, :US, :], in0=g[:, :US, :],
                    in1=attn[:, :US, None].to_broadcast([P, US, C]),
                    op=OP.mult)
                nc.vector.tensor_tensor(
                    out=wp3[:, US:, :], in0=g[:, US:, :],
                    in1=attn[:, US:, None].to_broadcast([P, CPP - US, C]),
                    op=OP.mult)

                # hctx: tree-halving over k (bf16 2x), final level emits f32
                w4 = wp[:].rearrange("p (s k c) -> p s k c", k=K, c=C)
                w = K // 2
                while w > 1:
                    nc.vector.tensor_tensor(
                        out=w4[:, :, :w, :], in0=w4[:, :, :w, :],
                        in1=w4[:, :, w:2 * w, :], op=OP.add)
                    w //= 2
                hctx = sml.tile([P, CSUP * C], f32, tag="hctx")
                nc.vector.tensor_tensor(
                    out=hctx[:].rearrange("p (s c) -> p s c", c=C)[:, :, None, :],
                    in0=w4[:, :, 0:1, :], in1=w4[:, :, 1:2, :], op=OP.add)

                pl = psL.tile([P, CSUP * A], f32, tag="pl", space="PSUM")
                for gi in range(CSUP):
                    s = CSUP * j + gi
                    pt = psm.tile([P, P], f32, tag="mix", space="PSUM")
                    nc.tensor.transpose(
                        out=pt[:], in_=hctx[:, gi * C:(gi + 1) * C],
                        identity=ident[:])
                    hctxT = sml.tile([P, P], f32, tag="hctxT")
                    nc.scalar.copy(out=hctxT[:], in_=pt[:])

                    sl = pl[:, gi * A:(gi + 1) * A]
                    nc.tensor.matmul(out=sl, lhsT=hidT[:, s * P:(s + 1) * P],
                                     rhs=decw_sb[:], start=True, stop=False)
                    nc.tensor.matmul(out=sl, lhsT=hctxT[:], rhs=w2_sb[:],
                                     start=False, stop=False)
                    nc.tensor.matmul(out=sl, lhsT=ones1[:], rhs=b2_sb[:],
                                     start=False, stop=True)

                el = sml.tile([P, CSUP * A], f32, tag="el")
                nc.scalar.activation(out=el[:], in_=pl[:], func=AF.Exp)
                zl = sml.tile([P, CSUP], f32, tag="zl")
                nc.vector.reduce_sum(
                    out=zl[:], in_=el[:].rearrange("p (s a) -> p s a", a=A),
                    axis=AX.X)
                rzl = sml.tile([P, CSUP], f32, tag="rzl")
                nc.vector.reciprocal(out=rzl[:], in_=zl[:])
                pr = sml.tile([P, CSUP * A], f32, tag="pr")
                nc.gpsimd.tensor_tensor(
                    out=pr[:].rearrange("p (s a) -> p s a", a=A),
                    in0=el[:].rearrange("p (s a) -> p s a", a=A),
                    in1=rzl[:, :, None].to_broadcast([P, CSUP, A]), op=OP.mult)

                nc.sync.dma_start(
                    out=outp[:, j * CSUP * A:(j + 1) * CSUP * A], in_=pr[:])

    nc.compile()
    return nc


def _fold_weights(enc_w, enc_b, msg_w, msg_b, key_w, key_b,
                  in_proj_w, in_proj_b, out_w, out_b, dec_w, dec_b):
    wq, wk, wv = in_proj_w[:C], in_proj_w[C:2 * C], in_proj_w[2 * C:]
    bq, bv = in_proj_b[:C], in_proj_b[2 * C:]
    Wq_eff = msg_w @ wq.T
    bq_eff = msg_b @ wq.T + bq
    Wk_eff = key_w @ wk.T
    Wv_eff = msg_w @ wv.T
    bv_eff = msg_b @ wv.T + bv
    s = np.float32(1.0 / np.sqrt(np.float32(C)))
    Wq2 = (Wq_eff @ Wk_eff.T) * s
    bq2 = (bq_eff @ Wk_eff.T) * s
    W2 = Wv_eff @ out_w @ dec_w
    b2 = bv_eff @ out_w @ dec_w + out_b @ dec_w + dec_b
    enc_w65 = np.concatenate([enc_w, enc_b[None, :]], axis=0)
    return enc_w65.astype(np.float32), Wq2.astype(np.float32), \
        bq2.astype(np.float32), W2.astype(np.float32), b2.astype(np.float32)


def _prep_in_maps(obs, neighbor_idx, enc_w, enc_b, msg_w, msg_b, key_w,
                  key_b, in_proj_w, in_proj_b, out_w, out_b, dec_w, dec_b):
    import ml_dtypes

    bf = ml_dtypes.bfloat16
    obs = np.asarray(obs, dtype=np.float32)
    idx = np.asarray(neighbor_idx).astype(np.int64)

    enc_w65, Wq2, bq2, W2, b2 = _fold_weights(
        np.asarray(enc_w, np.float32), np.asarray(enc_b, np.float32),
        np.asarray(msg_w, np.float32), np.asarray(msg_b, np.float32),
        np.asarray(key_w, np.float32), np.asarray(key_b, np.float32),
        np.asarray(in_proj_w, np.float32), np.asarray(in_proj_b, np.float32),
        np.asarray(out_w, np.float32), np.asarray(out_b, np.float32),
        np.asarray(dec_w, np.float32), np.asarray(dec_b, np.float32))

    obs_b = obs.astype(bf)          # bf16 copy for the edge path
    ones_col = np.ones((1,), bf)

    in_maps = []
    for c in range(NCORES):
        base = c * SHARD
        obsT_shard = np.zeros((IN_DIM + 1, NS), np.float32)
        obsT_shard[:IN_DIM, :SHARD] = obs[base:base + SHARD].T
        obsT_shard[IN_DIM, :] = 1.0

        # per-edge obs gather, laid out so that column (j*EPC + u*128 + p)
        # holds obs[idx[node(j,g,p), k]] with u = g*16+k, node = (j*CSUP+g)*128+p
        sh_idx = np.zeros((NS, K), np.int64)
        sh_idx[:SHARD] = idx[base:base + SHARD]
        # cols as [j, g, k, p] -> value idx[(j*CSUP+g)*128+p, k]
        idx_r = sh_idx.reshape(NCHUNK, CSUP, P, K)          # [j, g, p, k]
        col_idx = idx_r.transpose(0, 1, 3, 2).reshape(-1)   # [j, g, k, p]
        og = obs_b[col_idx]                                 # [NS*K, 64] bf16
        obsgT = np.empty((IN_DIM + 1, NS * K), bf)
        obsgT[:IN_DIM] = og.T
        obsgT[IN_DIM] = ones_col

        in_maps.append({
            "obsgT": obsgT, "obsT_shard": obsT_shard,
            "enc_wb": enc_w65.astype(bf), "enc_wf": enc_w65,
            "wq2": Wq2, "bq2row": bq2[None, :],
            "dec_w": np.asarray(dec_w, np.float32), "w2": W2,
            "b2row": b2[None, :],
        })
    return in_maps


def kernel(obs, neighbor_idx, enc_w, enc_b, msg_w, msg_b, key_w, key_b,
           in_proj_w, in_proj_b, out_w, out_b, dec_w, dec_b):
    from concourse import bass_utils

    in_maps = _prep_in_maps(
        obs, neighbor_idx, enc_w, enc_b, msg_w, msg_b, key_w, key_b,
        in_proj_w, in_proj_b, out_w, out_b, dec_w, dec_b)

    if "nc" not in _PROG_CACHE:
        _PROG_CACHE["nc"] = _build_program()
    nc = _PROG_CACHE["nc"]

    trace = bool(globals().get("_TRACE_RUN", False))
    res = bass_utils.run_bass_kernel_spmd(nc, in_maps, list(range(NCORES)),
                                          trace=trace)
    if trace:
        _PROG_CACHE["last_result"] = res

    out = np.empty((N, A), np.float32)
    for c in range(NCORES):
        o = res.results[c]["outp"].reshape(P, NSUP, A).transpose(1, 0, 2)
        out[c * SHARD:(c + 1) * SHARD] = o.reshape(NS, A)[:SHARD]
    return out



# revision 15
# speedup vs baseline: 1.0446x; 1.0446x over previous
"""AgentNet GNN message-passing kernel for 8 Trainium2 NeuronCores.

Algebraic collapse (validated to 4e-7 rel err vs reference in fp32,
~2e-3 with the bf16 paths used here):
  hidden = relu(obs @ enc_w + enc_b)                      [N,128]
  q'     = hidden @ Wq2 + bq2      (q/k projections folded; key bias
                                    cancels in softmax)   [N,128]
  s[n,k] = q'[n] . hidden[idx[n,k]]
  attn   = softmax_k(s)            (scores are O(0.05) -> no max-subtract)
  hctx   = sum_k attn[n,k] * hidden[idx[n,k]]
  logits = hidden @ dec_w + hctx @ W2 + b2
  out    = softmax_a(logits)

Key transformation: the neighbor gather commutes with the row-wise encoder,
so hidden[idx[n,k]] = relu(obs[idx[n,k]] @ enc_w + enc_b). The host gathers
obs rows per edge while sharding (cheap numpy take) and the device runs a
dense per-edge encoder matmul — no data-dependent DMA on device at all.

Engine budget (per chunk of 2 supertiles = 4096 edges): the elementwise
attention math (~16k bf16 rows) is split DVE/Pool by the *_DVE constants
below; Act owns relu + exp; all PE matmuls are bf16 (fp32 is 4 cyc/row).

Sharding: nodes split into 8 contiguous shards (6250/core, padded to 6656).
Weights replicated; no collectives.
"""
import numpy as np

N, K = 50000, 16
IN_DIM, H, C, A = 64, 128, 128, 32
NCORES = 8
SHARD = N // NCORES              # 6250
P = 128
NSUP = 52                        # supertiles of 128 nodes per core (6656)
NS = NSUP * P                    # 6656 padded shard nodes
CSUP = 4                         # supertiles per chunk
NCHUNK = NSUP // CSUP            # 26
EPC = CSUP * P * K               # 4096 edges per chunk
CPP = EPC // P                   # 32 slots per partition per chunk

# ---- engine split tuning (slots out of 32 that go to DVE; rest Pool) ----
# Broadcast-operand TTs (prod, wp) must stay on DVE: Pool gets no 2-byte
# fast path for stride-0 APs (~2 ns/row vs DVE 0.52).
PROD_DVE = 54        # of CPP slots of the score product
TREE1_DVE = 50       # of CPP slots of score-tree level 1 (64 wide)
TREE2_DVE = 50       # of CPP slots of score-tree level 2 (32 wide)
WP_DVE = 54          # of CPP slots of the attn weighting
# hctx tree level splits (k-pairs out of 8 at L1, 4 at L2 that go to DVE)
HT1_DVE = 6
HT2_DVE = 3
# relu granules (4 per chunk of [128,1024]): engine per granule
RELU_ENG = ("act",) * 8

_PROG_CACHE = {}


def _build_program():
    import concourse.bacc as bacc
    import concourse.mybir as mybir
    import concourse.tile as tile
    from concourse.masks import make_identity

    f32 = mybir.dt.float32
    bf16 = mybir.dt.bfloat16
    AX = mybir.AxisListType
    OP = mybir.AluOpType
    AF = mybir.ActivationFunctionType

    nc = bacc.Bacc("TRN2", target_bir_lowering=False, debug=False,
                   num_devices=1)

    # per-edge pre-gathered obs, transposed, with a trailing ones-row for the
    # encoder bias fold; column order: chunk j | slot u = g*16+k | partition p
    obsgT = nc.dram_tensor("obsgT", [IN_DIM + 1, NS * K], bf16, kind="ExternalInput")
    obsT_shard = nc.dram_tensor("obsT_shard", [IN_DIM + 1, NS], bf16, kind="ExternalInput")
    enc_wb = nc.dram_tensor("enc_wb", [IN_DIM + 1, H], bf16, kind="ExternalInput")
    wq2 = nc.dram_tensor("wq2", [H, C], bf16, kind="ExternalInput")
    bq2row = nc.dram_tensor("bq2row", [1, C], bf16, kind="ExternalInput")
    dec_w = nc.dram_tensor("dec_w", [H, A], bf16, kind="ExternalInput")
    w2 = nc.dram_tensor("w2", [C, A], bf16, kind="ExternalInput")
    b2row = nc.dram_tensor("b2row", [1, A], bf16, kind="ExternalInput")
    outp = nc.dram_tensor("outp", [P, NSUP * A], bf16, kind="ExternalOutput")

    with tile.TileContext(nc) as tc:
        with tc.tile_pool(name="cst", bufs=1) as cst, \
             tc.tile_pool(name="obs", bufs=2) as obsp, \
             tc.tile_pool(name="psA", bufs=2, space="PSUM") as psA, \
             tc.tile_pool(name="big", bufs=1) as big, \
             tc.tile_pool(name="psm", bufs=2, space="PSUM") as psm, \
             tc.tile_pool(name="gat", bufs=3) as gat, \
             tc.tile_pool(name="pw", bufs=3) as pwp, \
             tc.tile_pool(name="sml", bufs=3) as sml, \
             tc.tile_pool(name="psL", bufs=2, space="PSUM") as psL:

            # ---- constants ----
            encwb_sb = cst.tile([IN_DIM + 1, H], bf16)
            nc.sync.dma_start(out=encwb_sb[:], in_=enc_wb[:, :])
            wq2_sb = cst.tile([H, C], bf16)
            nc.sync.dma_start(out=wq2_sb[:], in_=wq2[:, :])
            bq2_sb = cst.tile([1, C], bf16)
            nc.sync.dma_start(out=bq2_sb[:], in_=bq2row[:, :])
            decw_sb = cst.tile([H, A], bf16)
            nc.sync.dma_start(out=decw_sb[:], in_=dec_w[:, :])
            w2_sb = cst.tile([C, A], bf16)
            nc.sync.dma_start(out=w2_sb[:], in_=w2[:, :])
            b2_sb = cst.tile([1, A], bf16)
            nc.sync.dma_start(out=b2_sb[:], in_=b2row[:, :])
            ones1 = cst.tile([1, P], bf16)
            nc.vector.memset(ones1[:], 1.0)
            ident = cst.tile([P, P], bf16)
            make_identity(nc, ident[:])
            obss_sb = cst.tile([IN_DIM + 1, NS], bf16)
            nc.sync.dma_start(out=obss_sb[:], in_=obsT_shard[:, :])

            # ---- stage B: shard hiddenT (bf16) and q' (bf16) ----
            hidT = big.tile([H, NS], bf16)
            for cb in range(NS // 512):
                ph = psm.tile([H, 512], f32, tag="mix", space="PSUM")
                nc.tensor.matmul(
                    out=ph[:], lhsT=encwb_sb[:],
                    rhs=obss_sb[:, cb * 512:(cb + 1) * 512],
                    start=True, stop=True)
                dst = hidT[:, cb * 512:(cb + 1) * 512]
                if cb % 3 == 1:   # GPSIMD cannot read PSUM on HW
                    nc.vector.tensor_scalar_max(out=dst, in0=ph[:], scalar1=0.0)
                else:
                    nc.scalar.activation(out=dst, in_=ph[:], func=AF.Relu)

            qp = big.tile([P, NSUP * C], bf16)
            for qb4 in range(NSUP // 4):
                pq = psm.tile([P, 512], f32, tag="mix", space="PSUM")
                for t in range(4):
                    s = qb4 * 4 + t
                    sl = pq[:, t * C:(t + 1) * C]
                    nc.tensor.matmul(out=sl, lhsT=hidT[:, s * P:(s + 1) * P],
                                     rhs=wq2_sb[:], start=True, stop=False)
                    nc.tensor.matmul(out=sl, lhsT=ones1[:], rhs=bq2_sb[:],
                                     start=False, stop=True)
                dst = qp[:, qb4 * 512:(qb4 + 1) * 512]
                if qb4 % 3 == 1:
                    nc.vector.tensor_scalar_mul(out=dst, in0=pq[:], scalar1=1.0)
                else:
                    nc.scalar.copy(out=dst, in_=pq[:])

            # ---- stage C: per-edge encoder + attention + decode ----
            qp4 = qp[:].rearrange("p (s c) -> p s c", c=C)
            for j in range(NCHUNK):
                og = obsp.tile([IN_DIM + 1, EPC], bf16, tag="og")
                nc.sync.dma_start(
                    out=og[:, :EPC // 2],
                    in_=obsgT[:, j * EPC:j * EPC + EPC // 2])
                nc.sync.dma_start(
                    out=og[:, EPC // 2:],
                    in_=obsgT[:, j * EPC + EPC // 2:(j + 1) * EPC])

                g = gat.tile([P, CPP, H], bf16, tag="g")
                for u8 in range(CPP // 8):
                    pe = psA.tile([P, 8 * H], f32, tag="pe", space="PSUM")
                    for t in range(8):
                        u = u8 * 8 + t
                        nc.tensor.matmul(
                            out=pe[:, t * H:(t + 1) * H],
                            lhsT=og[:, u * P:(u + 1) * P],
                            rhs=encwb_sb[:], start=True, stop=True)
                    dst = g[:, u8 * 8:(u8 + 1) * 8, :]
                    eng = RELU_ENG[u8]
                    if eng == "act":
                        nc.scalar.activation(out=dst, in_=pe[:], func=AF.Relu)
                    else:   # GPSIMD cannot read PSUM on HW
                        nc.vector.tensor_scalar_max(out=dst, in0=pe[:],
                                                    scalar1=0.0)

                g4 = g[:].rearrange("p (s k) c -> p s k c", k=K)
                qb = qp4[:, CSUP * j:CSUP * (j + 1), None, :].to_broadcast(
                    [P, CSUP, K, C])

                prod = pwp.tile([P, EPC], bf16, tag="pw")
                p3 = prod[:].rearrange("p (u c) -> p u c", c=C)
                pr4d = prod[:].rearrange("p (s k c) -> p s k c", k=K, c=C)
                g3 = g[:]
                # per-supertile 3D slices: a 4D slice with broadcast loses
                # the DVE 2x fast path (1.04 vs 0.52 ns/row)
                sf, ks = PROD_DVE // K, PROD_DVE % K
                for si in range(sf):
                    nc.vector.tensor_tensor(
                        out=pr4d[:, si, :, :], in0=g4[:, si, :, :],
                        in1=qb[:, si, :, :], op=OP.mult)
                if ks:
                    nc.vector.tensor_tensor(
                        out=pr4d[:, sf, :ks, :], in0=g4[:, sf, :ks, :],
                        in1=qb[:, sf, :ks, :], op=OP.mult)
                    nc.gpsimd.tensor_tensor(
                        out=pr4d[:, sf, ks:, :], in0=g4[:, sf, ks:, :],
                        in1=qb[:, sf, ks:, :], op=OP.mult)
                for si in range(sf + (1 if ks else 0), CSUP):
                    nc.gpsimd.tensor_tensor(
                        out=pr4d[:, si, :, :], in0=g4[:, si, :, :],
                        in1=qb[:, si, :, :], op=OP.mult)

                # scores: tree-halving TT adds over c (bf16 2x mode)
                u1 = TREE1_DVE
                if u1:
                    nc.vector.tensor_tensor(
                        out=p3[:, :u1, :64], in0=p3[:, :u1, :64],
                        in1=p3[:, :u1, 64:128], op=OP.add)
                if u1 < CPP:
                    nc.gpsimd.tensor_tensor(
                        out=p3[:, u1:, :64], in0=p3[:, u1:, :64],
                        in1=p3[:, u1:, 64:128], op=OP.add)
                u2 = TREE2_DVE
                if u2:
                    nc.vector.tensor_tensor(
                        out=p3[:, :u2, :32], in0=p3[:, :u2, :32],
                        in1=p3[:, :u2, 32:64], op=OP.add)
                if u2 < CPP:
                    nc.gpsimd.tensor_tensor(
                        out=p3[:, u2:, :32], in0=p3[:, u2:, :32],
                        in1=p3[:, u2:, 32:64], op=OP.add)
                w = C // 8
                while w > 1:
                    nc.vector.tensor_tensor(
                        out=p3[:, :, :w], in0=p3[:, :, :w],
                        in1=p3[:, :, w:2 * w], op=OP.add)
                    w //= 2
                sc = sml.tile([P, CSUP * K], f32, tag="sc")
                nc.vector.tensor_tensor(
                    out=sc[:, :, None], in0=p3[:, :, 0:1], in1=p3[:, :, 1:2],
                    op=OP.add)

                esc = sml.tile([P, CSUP * K], bf16, tag="esc")
                nc.scalar.activation(out=esc[:], in_=sc[:], func=AF.Exp)

                z = sml.tile([P, CSUP], f32, tag="z")
                nc.vector.reduce_sum(
                    out=z[:], in_=esc[:].rearrange("p (s k) -> p s k", k=K),
                    axis=AX.X)
                rz = sml.tile([P, CSUP], f32, tag="rz")
                nc.vector.reciprocal(out=rz[:], in_=z[:])

                attn = sml.tile([P, CSUP * K], bf16, tag="attn")
                for gi in range(CSUP):
                    nc.vector.tensor_scalar_mul(
                        out=attn[:, gi * K:(gi + 1) * K],
                        in0=esc[:, gi * K:(gi + 1) * K],
                        scalar1=rz[:, gi:gi + 1])

                wp = pwp.tile([P, EPC], bf16, tag="pw")
                wp3 = wp[:].rearrange("p (u c) -> p u c", c=C)
                us = WP_DVE
                nc.vector.tensor_tensor(
                    out=wp3[:, :us, :], in0=g3[:, :us, :],
                    in1=attn[:, :us, None].to_broadcast([P, us, C]),
                    op=OP.mult)
                if us < CPP:
                    nc.gpsimd.tensor_tensor(
                        out=wp3[:, us:, :], in0=g3[:, us:, :],
                        in1=attn[:, us:, None].to_broadcast([P, CPP - us, C]),
                        op=OP.mult)

                # hctx: tree-halving over k (bf16 2x), split DVE/Pool
                w4 = wp[:].rearrange("p (s k c) -> p s k c", k=K, c=C)
                h1 = HT1_DVE
                if h1:
                    nc.vector.tensor_tensor(
                        out=w4[:, :, :h1, :], in0=w4[:, :, :h1, :],
                        in1=w4[:, :, 8:8 + h1, :], op=OP.add)
                if h1 < 8:
                    nc.gpsimd.tensor_tensor(
                        out=w4[:, :, h1:8, :], in0=w4[:, :, h1:8, :],
                        in1=w4[:, :, 8 + h1:16, :], op=OP.add)
                h2 = HT2_DVE
                if h2:
                    nc.vector.tensor_tensor(
                        out=w4[:, :, :h2, :], in0=w4[:, :, :h2, :],
                        in1=w4[:, :, 4:4 + h2, :], op=OP.add)
                if h2 < 4:
                    nc.gpsimd.tensor_tensor(
                        out=w4[:, :, h2:4, :], in0=w4[:, :, h2:4, :],
                        in1=w4[:, :, 4 + h2:8, :], op=OP.add)
                nc.vector.tensor_tensor(
                    out=w4[:, :, :2, :], in0=w4[:, :, :2, :],
                    in1=w4[:, :, 2:4, :], op=OP.add)
                hctx = sml.tile([P, CSUP * C], bf16, tag="hctx")
                nc.vector.tensor_tensor(
                    out=hctx[:].rearrange("p (s c) -> p s c", c=C)[:, :, None, :],
                    in0=w4[:, :, 0:1, :], in1=w4[:, :, 1:2, :], op=OP.add)

                pl = psL.tile([P, CSUP * A], f32, tag="pl", space="PSUM")
                pt = psm.tile([P, CSUP * P], bf16, tag="mix", space="PSUM")
                hctxT = sml.tile([P, CSUP * P], bf16, tag="hctxT")
                for gi in range(CSUP):
                    nc.tensor.transpose(
                        out=pt[:, gi * P:(gi + 1) * P],
                        in_=hctx[:, gi * C:(gi + 1) * C],
                        identity=ident[:])
                nc.scalar.copy(out=hctxT[:], in_=pt[:])
                for gi in range(CSUP):
                    s = CSUP * j + gi
                    sl = pl[:, gi * A:(gi + 1) * A]
                    nc.tensor.matmul(out=sl, lhsT=hidT[:, s * P:(s + 1) * P],
                                     rhs=decw_sb[:], start=True, stop=False)
                    nc.tensor.matmul(out=sl, lhsT=hctxT[:, gi * P:(gi + 1) * P],
                                     rhs=w2_sb[:], start=False, stop=False)
                    nc.tensor.matmul(out=sl, lhsT=ones1[:], rhs=b2_sb[:],
                                     start=False, stop=True)

                el = sml.tile([P, CSUP * A], bf16, tag="el")
                nc.scalar.activation(out=el[:], in_=pl[:], func=AF.Exp)
                zl = sml.tile([P, CSUP], f32, tag="zl")
                nc.vector.reduce_sum(
                    out=zl[:], in_=el[:].rearrange("p (s a) -> p s a", a=A),
                    axis=AX.X)
                rzl = sml.tile([P, CSUP], f32, tag="rzl")
                nc.vector.reciprocal(out=rzl[:], in_=zl[:])
                pr = sml.tile([P, CSUP * A], bf16, tag="pr")
                for gi in range(CSUP):
                    nc.vector.tensor_scalar_mul(
                        out=pr[:, gi * A:(gi + 1) * A],
                        in0=el[:, gi * A:(gi + 1) * A],
                        scalar1=rzl[:, gi:gi + 1])

                nc.sync.dma_start(
                    out=outp[:, j * CSUP * A:(j + 1) * CSUP * A], in_=pr[:])

    nc.compile()
    return nc


def _fold_weights(enc_w, enc_b, msg_w, msg_b, key_w, key_b,
                  in_proj_w, in_proj_b, out_w, out_b, dec_w, dec_b):
    wq, wk, wv = in_proj_w[:C], in_proj_w[C:2 * C], in_proj_w[2 * C:]
    bq, bv = in_proj_b[:C], in_proj_b[2 * C:]
    Wq_eff = msg_w @ wq.T
    bq_eff = msg_b @ wq.T + bq
    Wk_eff = key_w @ wk.T
    Wv_eff = msg_w @ wv.T
    bv_eff = msg_b @ wv.T + bv
    s = np.float32(1.0 / np.sqrt(np.float32(C)))
    Wq2 = (Wq_eff @ Wk_eff.T) * s
    bq2 = (bq_eff @ Wk_eff.T) * s
    W2 = Wv_eff @ out_w @ dec_w
    b2 = bv_eff @ out_w @ dec_w + out_b @ dec_w + dec_b
    enc_w65 = np.concatenate([enc_w, enc_b[None, :]], axis=0)
    return enc_w65.astype(np.float32), Wq2.astype(np.float32), \
        bq2.astype(np.float32), W2.astype(np.float32), b2.astype(np.float32)


def _prep_in_maps(obs, neighbor_idx, enc_w, enc_b, msg_w, msg_b, key_w,
                  key_b, in_proj_w, in_proj_b, out_w, out_b, dec_w, dec_b):
    import ml_dtypes

    bf = ml_dtypes.bfloat16
    obs = np.asarray(obs, dtype=np.float32)
    idx = np.asarray(neighbor_idx).astype(np.int64)

    enc_w65, Wq2, bq2, W2, b2 = _fold_weights(
        np.asarray(enc_w, np.float32), np.asarray(enc_b, np.float32),
        np.asarray(msg_w, np.float32), np.asarray(msg_b, np.float32),
        np.asarray(key_w, np.float32), np.asarray(key_b, np.float32),
        np.asarray(in_proj_w, np.float32), np.asarray(in_proj_b, np.float32),
        np.asarray(out_w, np.float32), np.asarray(out_b, np.float32),
        np.asarray(dec_w, np.float32), np.asarray(dec_b, np.float32))

    obs_b = obs.astype(bf)          # bf16 copy for the edge path
    ones_col = np.ones((1,), bf)

    in_maps = []
    for c in range(NCORES):
        base = c * SHARD
        obsT_shard = np.zeros((IN_DIM + 1, NS), bf)
        obsT_shard[:IN_DIM, :SHARD] = obs_b[base:base + SHARD].T
        obsT_shard[IN_DIM, :] = ones_col

        # per-edge obs gather, laid out so that column (j*EPC + u*128 + p)
        # holds obs[idx[node(j,g,p), k]] with u = g*16+k, node = (j*CSUP+g)*128+p
        sh_idx = np.zeros((NS, K), np.int64)
        sh_idx[:SHARD] = idx[base:base + SHARD]
        # cols as [j, g, k, p] -> value idx[(j*CSUP+g)*128+p, k]
        idx_r = sh_idx.reshape(NCHUNK, CSUP, P, K)          # [j, g, p, k]
        col_idx = idx_r.transpose(0, 1, 3, 2).reshape(-1)   # [j, g, k, p]
        og = obs_b[col_idx]                                 # [NS*K, 64] bf16
        obsgT = np.empty((IN_DIM + 1, NS * K), bf)
        obsgT[:IN_DIM] = og.T
        obsgT[IN_DIM] = ones_col

        in_maps.append({
            "obsgT": obsgT, "obsT_shard": obsT_shard,
            "enc_wb": enc_w65.astype(bf),
            "wq2": Wq2.astype(bf), "bq2row": bq2[None, :].astype(bf),
            "dec_w": np.asarray(dec_w, np.float32).astype(bf),
            "w2": W2.astype(bf), "b2row": b2[None, :].astype(bf),
        })
    return in_maps


def kernel(obs, neighbor_idx, enc_w, enc_b, msg_w, msg_b, key_w, key_b,
           in_proj_w, in_proj_b, out_w, out_b, dec_w, dec_b):
    from concourse import bass_utils

    in_maps = _prep_in_maps(
        obs, neighbor_idx, enc_w, enc_b, msg_w, msg_b, key_w, key_b,
        in_proj_w, in_proj_b, out_w, out_b, dec_w, dec_b)

    if "nc" not in _PROG_CACHE:
        _PROG_CACHE["nc"] = _build_program()
    nc = _PROG_CACHE["nc"]

    trace = bool(globals().get("_TRACE_RUN", False))
    res = bass_utils.run_bass_kernel_spmd(nc, in_maps, list(range(NCORES)),
                                          trace=trace)
    if trace:
        _PROG_CACHE["last_result"] = res

    out = np.empty((N, A), np.float32)
    for c in range(NCORES):
        o = np.asarray(res.results[c]["outp"], dtype=np.float32)
        o = o.reshape(P, NSUP, A).transpose(1, 0, 2)
        out[c * SHARD:(c + 1) * SHARD] = o.reshape(NS, A)[:SHARD]
    return out
